# revision 7
# baseline (speedup 1.0000x reference)
"""Trainium2 Bass kernel for nn_Net_49950469652573 — dead-code-eliminated.

Architectural invariant: the SConv2dLSTM membrane is mem1 = sigmoid(go) *
tanh(syn1), which is <= 1 = THRESH for every possible input (including
+/-Inf, where the product saturates at exactly 1.0, still failing the
strict > comparison). Hence spk1 = (maxpool(mem1) - 1 > 0) == 0
unconditionally, cur2 = spk1 @ fc1_w.T + fc1_b == fc1_b, and the entire
ConvLSTM + fc1 matmul pipeline is dead code: the model output depends only
on fc1_b and the CfC weights/biases/masks. Because cur2 is also
batch-independent, every sample's mem2/spk2 trajectory is identical, so no
cross-core communication is needed either; the remaining computation is a
75-wide Leaky scan, the 3-layer CfC scan threaded over the 8 batch
elements (t batched in the matmul free dim), and the 6-wide mem3 Leaky.

This kernel computes those recurrences faithfully from the runtime inputs
(it does NOT hardcode the zero outputs), so it remains exact for arbitrary
values of every input tensor. All constants arrive in one packed DMA; the
CfC keeps per-layer state in base-0 tiles and contracts over (state,
input) with two accumulating matmuls, so no partition-moving DMA copies
sit on the scan's critical path. Runs on a single NeuronCore.
"""

import numpy as np

import concourse.bacc as bacc
import concourse.bass as bass
import concourse.tile as tile
from concourse import mybir
from concourse.bass_utils import run_bass_kernel_spmd

F32 = mybir.dt.float32
AOP = mybir.AluOpType
AF = mybir.ActivationFunctionType

T = 16
BETA = 0.9

# per-layer (input_rows, hidden): l0 input = spk2 (75), l1 input = h0' (9),
# l2 input = h1' (5)
LAY = [(75, 9), (9, 5), (5, 6)]


def _pack_cols():
    cols = {}
    c = 0
    for l, (din, h) in enumerate(LAY):
        for nm in ("f1", "f2", "ta", "tb", "mk"):
            for piece in ("h", "x"):
                cols[(l, nm, piece)] = (c, h)
                c += h
        cols[(l, "cb")] = (c, 4)
        c += 4
    cols["fc1b"] = (c, 1)
    c += 1
    return cols, c


PCOLS, PWIDTH = _pack_cols()


def build(reps=1, sim=False):
    nc = bacc.Bacc("TRN2", target_bir_lowering=False, debug=False,
                   num_devices=1)

    pack_d = nc.dram_tensor("pack", [75, PWIDTH], F32, kind="ExternalInput")
    ospk = nc.dram_tensor("ospk", [T * 8 * 6], F32, kind="ExternalOutput")
    omem = nc.dram_tensor("omem", [T * 8 * 6], F32, kind="ExternalOutput")

    with tile.TileContext(nc) as tc:
        with (
            tc.tile_pool(name="persist", bufs=1) as pp,
            tc.tile_pool(name="work", bufs=3) as wk,
            tc.tile_pool(name="psum", bufs=8, space="PSUM") as psp,
        ):
            pack = pp.tile([75, PWIDTH], F32)
            mem2 = pp.tile([75, 1], F32)
            spk2 = pp.tile([75, 16], F32)
            zero75 = pp.tile([75, 1], F32)
            zero6 = pp.tile([6, 8], F32)
            W = {}
            for l, (din, h) in enumerate(LAY):
                for piece, rows in (("h", h), ("x", din)):
                    W[(l, "wf1", piece)] = pp.tile([rows, h], F32,
                                                   name=f"wf1{l}{piece}")
                    W[(l, "wf2", piece)] = pp.tile([rows, h], F32,
                                                   name=f"wf2{l}{piece}")
                    W[(l, "wt", piece)] = pp.tile([rows, h], F32,
                                                  name=f"wt{l}{piece}")
                W[(l, "btt")] = pp.tile([h, 1], F32, name=f"btt{l}")
            hs = [pp.tile([h, 16], F32, name=f"hs{l}")
                  for l, (_, h) in enumerate(LAY)]
            cur3 = pp.tile([6, 8 * 16], F32)
            om = pp.tile([6, T * 8], F32)
            osb = pp.tile([6, T * 8], F32)

            # ---------------- load + prep constants ----------------
            nc.sync.dma_start(out=pack[:], in_=pack_d[:])

            def pc(l, nm, piece=None):
                key = (l, nm) if piece is None else (l, nm, piece)
                c, w = PCOLS[key]
                if piece == "x":
                    rows = LAY[l][0]
                else:
                    rows = LAY[l][1]
                return pack[0:rows, c:c + w]

            for l, (din, h) in enumerate(LAY):
                for piece in ("h", "x"):
                    nc.vector.tensor_mul(W[(l, "wf1", piece)][:],
                                         pc(l, "f1", piece), pc(l, "mk", piece))
                    nc.vector.tensor_mul(W[(l, "wf2", piece)][:],
                                         pc(l, "f2", piece), pc(l, "mk", piece))
                    nc.vector.tensor_add(W[(l, "wt", piece)][:],
                                         pc(l, "ta", piece), pc(l, "tb", piece))
                nc.vector.tensor_add(W[(l, "btt")][:],
                                     pc(l, "cb")[:, 2:3], pc(l, "cb")[:, 3:4])
            fb0 = PCOLS["fc1b"][0]
            fc1b = pack[0:75, fb0:fb0 + 1]

            for rep in range(reps):
                nc.vector.memset(mem2[:], 0.0)
                nc.vector.memset(zero75[:], 0.0)
                nc.vector.memset(zero6[:], 0.0)
                for l in range(3):
                    nc.vector.memset(hs[l][:], 0.0)

                # mem2/spk2 Leaky scan: cur2(t) == fc1_b for every t, sample
                r2t = wk.tile([75, 1], F32, tag="r2t")
                for t in range(T):
                    prev = zero75[:] if t == 0 else spk2[:, t - 1:t]
                    nc.vector.tensor_sub(r2t[:], fc1b, prev)
                    nc.vector.tensor_scalar_mul(mem2[:], mem2[:], BETA)
                    nc.vector.tensor_add(mem2[:], mem2[:], r2t[:])
                    nc.vector.tensor_scalar(out=spk2[:, t:t + 1], in0=mem2[:],
                                            scalar1=1.0, scalar2=None,
                                            op0=AOP.is_gt)

                # ---------------- CfC scan over batch ----------------
                # layer l at step b contracts [state_l ; input_l] via two
                # accumulating matmuls; input_0 = spk2 (same every b),
                # input_{l+1} = h_l after its in-place update.
                for b in range(8):
                    col = slice(b * 16, (b + 1) * 16)
                    for l, (din, h) in enumerate(LAY):
                        xin = spk2[:] if l == 0 else hs[l - 1][:]
                        pf1 = psp.tile([16, 16], F32, tag="ps")
                        pf2 = psp.tile([16, 16], F32, tag="ps")
                        pti = psp.tile([16, 16], F32, tag="ps")
                        for ps_, wn in ((pf1, "wf1"), (pf2, "wf2"),
                                        (pti, "wt")):
                            nc.tensor.matmul(ps_[0:h, :], W[(l, wn, "h")][:],
                                             hs[l][:], start=True, stop=False)
                            nc.tensor.matmul(ps_[0:h, :], W[(l, wn, "x")][:],
                                             xin, start=False, stop=True)
                        f1 = wk.tile([16, 16], F32, tag=f"f1_{l}")
                        f2 = wk.tile([16, 16], F32, tag=f"f2_{l}")
                        ti = wk.tile([16, 16], F32, tag=f"ti_{l}")
                        nc.scalar.activation(out=f1[0:h, :], in_=pf1[0:h, :],
                                             func=AF.Tanh,
                                             bias=pc(l, "cb")[:, 0:1])
                        nc.scalar.activation(out=f2[0:h, :], in_=pf2[0:h, :],
                                             func=AF.Tanh,
                                             bias=pc(l, "cb")[:, 1:2])
                        nc.scalar.activation(out=ti[0:h, :], in_=pti[0:h, :],
                                             func=AF.Sigmoid,
                                             bias=W[(l, "btt")][:])
                        # h' = f1 + ti*(f2 - f1), updated in place
                        nc.vector.tensor_sub(f2[0:h, :], f2[0:h, :],
                                             f1[0:h, :])
                        nc.vector.tensor_mul(f2[0:h, :], f2[0:h, :],
                                             ti[0:h, :])
                        nc.vector.tensor_add(hs[l][:], f1[0:h, :], f2[0:h, :])
                        if l == 2:
                            nc.vector.tensor_copy(out=cur3[:, col],
                                                  in_=hs[2][:])

                # mem3 Leaky over t: om holds mem3 history, osb the spikes
                c3v = cur3[:].rearrange("p (b t) -> p b t", t=16)
                r3t = wk.tile([6, 8], F32, tag="r3t")
                for t in range(T):
                    prev_s = zero6[:] if t == 0 else osb[:, (t - 1) * 8:t * 8]
                    nc.vector.tensor_sub(r3t[:], c3v[:, :, t], prev_s)
                    if t == 0:
                        nc.vector.tensor_copy(out=om[:, 0:8], in_=r3t[:])
                    else:
                        nc.vector.tensor_scalar(
                            out=om[:, t * 8:(t + 1) * 8],
                            in0=om[:, (t - 1) * 8:t * 8],
                            scalar1=BETA, scalar2=None, op0=AOP.mult)
                        nc.vector.tensor_add(om[:, t * 8:(t + 1) * 8],
                                             om[:, t * 8:(t + 1) * 8], r3t[:])
                    nc.vector.tensor_scalar(out=osb[:, t * 8:(t + 1) * 8],
                                            in0=om[:, t * 8:(t + 1) * 8],
                                            scalar1=1.0, scalar2=None,
                                            op0=AOP.is_gt)

                odst = [[1, 6], [48, T], [6, 8]]
                nc.sync.dma_start(out=bass.AP(tensor=omem, offset=0, ap=odst),
                                  in_=om[:])
                nc.scalar.dma_start(out=bass.AP(tensor=ospk, offset=0,
                                                ap=odst),
                                    in_=osb[:])

    if not sim:
        nc.compile()
    return nc


def _prep_shared(fc1_b, cws, cbs, masks):
    pack = np.zeros((75, PWIDTH), np.float32)
    # reference concat order is [inp, h]: piece 'x' = input-axis rows
    # 0:din, piece 'h' = rows din:din+h.
    for l, (din, h) in enumerate(LAY):
        w4 = np.asarray(cws[l], np.float32)       # [4, h, din+h]
        b4 = np.asarray(cbs[l], np.float32)       # [4, h]
        mk = np.asarray(masks[l], np.float32)     # [h, din+h]
        for nm, mat in (("f1", w4[0]), ("f2", w4[1]), ("ta", w4[2]),
                        ("tb", w4[3]), ("mk", mk)):
            for piece, sl, rows in (("x", slice(0, din), din),
                                    ("h", slice(din, din + h), h)):
                c, w = PCOLS[(l, nm, piece)]
                pack[0:rows, c:c + w] = mat[:, sl].T
        c, w = PCOLS[(l, "cb")]
        pack[0:h, c:c + w] = b4.T
    c, _ = PCOLS["fc1b"]
    pack[0:75, c] = np.asarray(fc1_b, np.float32).reshape(75)
    return {"pack": pack}


_CACHE = {}


def _get_nc(reps=1):
    if reps not in _CACHE:
        _CACHE[reps] = build(reps=reps)
    return _CACHE[reps]


def make_in_maps(inputs):
    shared = _prep_shared(
        inputs["fc1_b"],
        [inputs["cfc_w0"], inputs["cfc_w1"], inputs["cfc_w2"]],
        [inputs["cfc_b0"], inputs["cfc_b1"], inputs["cfc_b2"]],
        [inputs["mask0"], inputs["mask1"], inputs["mask2"]],
    )
    return [shared]


def kernel(**inputs):
    nc = _get_nc(reps=1)
    in_maps = make_in_maps(inputs)
    res = run_bass_kernel_spmd(nc, in_maps, [0])
    r0 = res.results[0]
    spk3 = r0["ospk"].reshape(T, 8, 6).astype(np.float32)
    mem3 = r0["omem"].reshape(T, 8, 6).astype(np.float32)
    return spk3, mem3
